# revision 23
# baseline (speedup 1.0000x reference)
"""Trainium2 Bass kernel for nn_Encoder_LSTM (4x LSTMCell with zero state over
packed ragged tokens).

Math (from the reference): all rows independent; for each layer:
    gates = x @ W_ih^T + (b_ih + b_hh);  i, f, g, o = split(gates)
    c = sigmoid(i) * tanh(g);  h = sigmoid(o) * tanh(c)      (f unused)
Outputs: (output=h4, h1, c1, h2, c2, h3, c3, h4, c4), each [sum(bs), 512] f32.

Device strategy (v2, layer-major pipelined):
  - Dedupe the ragged gather: only U=16448 distinct source rows; each core
    gets 2056 rows (round-robin), ordered cov-descending so stores are
    contiguous prefixes per (tile, dup-slot). Host scatters slab -> logical.
  - sigmoid(z) = 0.5*(1+tanh(z/2)): everything becomes tanh. The 0.5/2
    factors fold into the packed weights (W_io/2 for layers 2-4 whose input
    is h'=2h; 2*W_g so ONE tanh(0.5*z) op covers all 1536 gate columns) and
    a host-side 0.5 on all stored outputs (device stores h'=2h, c'=2c).
  - DVE adds the bias (PSUM -> bf16 SBUF), ACT runs one tanh over all
    gates, DVE computes c'=(ti+1)*tg and h'=(to+1)*tc via fast-mode
    tensor_scalar_add + tensor_mul, ACT does the PSUM->SBUF aT copies.
  - Layer 1 runs in f32r (full-speed fp32 matmul) with HOST-pretransposed x:
    no on-device transposes for layer 1 and much lower matmul error.
    Layers 2-4 run bf16 with PE transposes of h'.
  - Emission is one FLAT software pipeline over (layer, tile): matmuls of
    consecutive tiles run back-to-back on the PE (keeps its p-state at
    2.4GHz); h' transposes lag 2 steps; tanh(c')/h' lag 1 step.
  - (h',c') share one pair tile -> ONE store DMA per (tile, slot, layer),
    bf16 (host upconverts), layout hc[layer][row][h|c][512]. Store triggers
    split sync/gpsimd by slot; input DMAs sliced across engines so the
    first tile's weights/activations land in ~4us.
"""

import sys

if "/opt/trn_rl_repo" not in sys.path:
    sys.path.insert(0, "/opt/trn_rl_repo")

import numpy as np
import ml_dtypes

P = 128
H = 512
G = 1536          # 3 packed gates [i, o, g] * 512
J = 8             # fused outputs [h1, c1, h2, c2, h3, c3, h4, c4]
NCORES = 8
OUT_NAMES = ["h1", "c1", "h2", "c2", "h3", "c3", "h4", "c4"]


# ---------------------------------------------------------------- host plan

def _make_plan(batch_sizes):
    bs = np.asarray(batch_sizes).astype(np.int64)
    s = np.concatenate([i * b + np.arange(b) for i, b in enumerate(bs)]).astype(np.int64)
    Nout = int(s.size)
    U = int(s.max()) + 1
    cov = np.bincount(s, minlength=U)

    cores = []
    for c in range(NCORES):
        src = np.arange(c, U, NCORES, dtype=np.int64)
        order = np.argsort(-cov[src], kind="stable")
        src_o = src[order]                       # this core's rows, cov-desc
        out_js = np.flatnonzero((s % NCORES) == c)   # global out rows (asc)
        src_of_slab = s[out_js]
        sort_slab = np.argsort(src_of_slab, kind="stable")
        srcs_sorted = src_of_slab[sort_slab]
        lo = np.searchsorted(srcs_sorted, src_o, "left")
        hi = np.searchsorted(srcs_sorted, src_o, "right")
        cores.append(dict(src_o=src_o, out_js=out_js, sort_slab=sort_slab,
                          lo=lo, hi=hi, covs=(hi - lo)))

    n_src = [len(cc["src_o"]) for cc in cores]
    T_tiles = max((n + P - 1) // P for n in n_src)

    # per-(tile, slot) prefix length, uniform across cores (max)
    MT = []
    for t in range(T_tiles):
        K = 1
        for cc in cores:
            cv = cc["covs"][t * P:(t + 1) * P]
            if len(cv):
                K = max(K, int(cv.max()))
        ms = []
        for k in range(K):
            m = 0
            for cc in cores:
                cv = cc["covs"][t * P:(t + 1) * P]
                if len(cv):
                    m = max(m, int((cv > k).sum()))
            ms.append(m)
        MT.append(ms)
    B = []
    off = 0
    for ms in MT:
        Bs_ = []
        for m in ms:
            Bs_.append(off)
            off += m
        B.append(Bs_)
    O_alloc = off
    OBIG = J * O_alloc

    for cc in cores:
        slab_rows, glob_rows = [], []
        covs, lo, sort_slab = cc["covs"], cc["lo"], cc["sort_slab"]
        out_js = cc["out_js"]
        n = len(cc["src_o"])
        for t in range(T_tiles):
            for k, m in enumerate(MT[t]):
                base = B[t][k]
                for p in range(m):
                    r = t * P + p
                    if r < n and covs[r] > k:
                        slab_rows.append(base + p)
                        glob_rows.append(out_js[sort_slab[lo[r] + k]])
        cc["slab_rows"] = np.asarray(slab_rows, np.int64)
        cc["glob_rows"] = np.asarray(glob_rows, np.int64)

    return dict(s=s, Nout=Nout, U=U, cores=cores, T_tiles=T_tiles,
                MT=MT, B=B, O_alloc=O_alloc, OBIG=OBIG)


def _pack_weights(inputs):
    """All gates use one tanh(0.5*z') on-device:
       io columns carry z'_io = z_io (layer1) / h' @ (W_io/2) (layers 2-4,
       inputs are h'=2h);  g columns carry z'_g = 2*z_g.
    -> w1 [4,P,G] f32 = [W_io | 2*W_g]^T chunks (layer 1),
       wb [3,4,P,G] bf16 = [W_io/2 | W_g]^T chunks,
       br1 [1,G] f32 / brb [1,3G] bf16 bias rows = [b_io | 2*b_g]."""
    w1 = np.zeros((4, P, G), np.float32)
    wb = np.zeros((3, 4, P, G), ml_dtypes.bfloat16)
    br1 = np.zeros((1, G), np.float32)
    brb = np.zeros((1, 3 * G), ml_dtypes.bfloat16)
    brep = np.zeros((P, 4 * G), ml_dtypes.bfloat16)
    for li in range(4):
        W = np.asarray(inputs[f"W_ih{li+1}"], np.float32)        # [2048, 512]
        bb = (np.asarray(inputs[f"b_ih{li+1}"], np.float32)
              + np.asarray(inputs[f"b_hh{li+1}"], np.float32))   # [2048]
        Wigo = np.concatenate([W[0:512], W[1536:2048], W[1024:1536]], axis=0)
        bigo = np.concatenate([bb[0:512], bb[1536:2048], bb[1024:1536]])
        bigo = np.concatenate([bigo[0:1024], 2.0 * bigo[1024:1536]])
        WT = np.ascontiguousarray(Wigo.T).copy()                 # [512, 1536]
        if li == 0:
            WT[:, 1024:1536] *= 2.0
            for k in range(4):
                w1[k] = WT[k * P:(k + 1) * P]
            br1[0] = bigo
        else:
            WT[:, 0:1024] *= 0.5
            for k in range(4):
                wb[li - 1, k] = WT[k * P:(k + 1) * P].astype(ml_dtypes.bfloat16)
            brb[0, (li - 1) * G:li * G] = bigo.astype(ml_dtypes.bfloat16)
        brep[:, li * G:(li + 1) * G] = bigo.astype(ml_dtypes.bfloat16)[None, :]
    return w1, wb, br1, brb, brep


# ---------------------------------------------------------------- bass build

def _build_nc(T_tiles, MT, B, O_alloc, OBIG):
    import concourse.mybir as mybir
    from concourse import bacc
    from concourse.masks import make_identity
    from concourse.tile import TileContext

    dt = mybir.dt
    AF = mybir.ActivationFunctionType
    ALU = mybir.AluOpType
    NT = T_tiles

    nc = bacc.Bacc()
    # host-pretransposed x: [k-chunk, feat-in-chunk, token]
    xt_d = nc.dram_tensor("xt", [NT, P, 4 * P], dt.float32r, kind="ExternalInput")
    w1_d = nc.dram_tensor("w1", [4, P, G], dt.float32r, kind="ExternalInput")
    wb_d = nc.dram_tensor("wb", [3, 4, P, G], dt.bfloat16, kind="ExternalInput")
    br1_d = nc.dram_tensor("br1", [1, G], dt.float32r, kind="ExternalInput")
    brb_d = nc.dram_tensor("brb", [1, 3 * G], dt.bfloat16, kind="ExternalInput")
    brep_d = nc.dram_tensor("brep", [P, 4 * G], dt.bfloat16, kind="ExternalInput")
    o_d = nc.dram_tensor("hc", [4, O_alloc, 2, H], dt.bfloat16, kind="ExternalOutput")

    with TileContext(nc) as tc:
        with (
            tc.tile_pool(name="const", bufs=1) as constp,
            tc.tile_pool(name="aT", bufs=2) as aTp,       # lhsT ring (2/tag)
            tc.tile_pool(name="gsb", bufs=3) as gsbp,
            tc.tile_pool(name="tp1", bufs=3) as tp1p,
            tc.tile_pool(name="act", bufs=4) as actp,     # t_all/tcl
            tc.tile_pool(name="hcp", bufs=8) as hcp,      # (h',c') pair tiles
            tc.tile_pool(name="psg", bufs=2, space="PSUM") as psgp,
            tc.tile_pool(name="pst", bufs=2, space="PSUM") as pstp,
        ):
            # ---- constants / weights
            br1_sb = constp.tile([1, G], dt.float32r)
            nc.scalar.dma_start(br1_sb[:], br1_d[:])
            brb_sb = constp.tile([1, 3 * G], dt.bfloat16)
            nc.scalar.dma_start(brb_sb[:], brb_d[:])
            brep_sb = constp.tile([P, 4 * G], dt.bfloat16)
            nc.scalar.dma_start(brep_sb[:], brep_d[:])
            one1 = constp.tile([1, P], dt.float32r)
            nc.vector.memset(one1[:].bitcast(dt.float32), 1.0)
            oneb = constp.tile([1, P], dt.bfloat16)
            nc.vector.memset(oneb[:], 1.0)
            id_bf = constp.tile([P, P], dt.bfloat16)
            make_identity(nc, id_bf[:])

            # layer-1 stationary tiles first (xa[0] unblocks the pipeline),
            # interleaved with weights, spread across issuing engines
            xa = []
            for t in range(NT):
                x_t = constp.tile([P, 4 * P], dt.float32r, name=f"xa{t}",
                                  tag=f"xa{t}")
                xa.append(x_t)
            w1_sb = constp.tile([P, 4 * G], dt.float32r)
            wb_sb = constp.tile([P, 12 * G], dt.bfloat16)
            engs = [nc.scalar, nc.sync, nc.gpsimd]
            for t in (0, 1):
                for k in range(4):
                    engs[k % 3].dma_start(xa[t][:, k * P:(k + 1) * P],
                                          xt_d[t, :, k * P:(k + 1) * P])
            ei = 0
            for n in range(3):           # region-major: n=0 slices land first
                for k in range(4):
                    engs[ei % 3].dma_start(
                        w1_sb[:, k * G + n * H:k * G + (n + 1) * H],
                        w1_d[k, :, n * H:(n + 1) * H])
                    ei += 1
            for t in range(2, NT):
                [nc.scalar, nc.gpsimd][t % 2].dma_start(xa[t][:], xt_d[t])
            for li in range(3):
                for k in range(4):
                    j = li * 4 + k
                    [nc.gpsimd, nc.scalar][j % 2].dma_start(
                        wb_sb[:, j * G:(j + 1) * G], wb_d[li, k])

            aT = [None] * NT     # per-tile lhsT for current layer >= 2
            hprev = [None] * NT  # h' tile pending transpose

            def emit_mm(li, t, use_opener):
                """12 accumulating matmuls; optional PE bias-opener (unused
                in the final config: DVE bias-add balances better)."""
                g_ps = psgp.tile([P, G], dt.float32, tag="psg", name="g_ps")
                if use_opener:
                    if li == 0:
                        one, brow, bbase = one1, br1_sb, 0
                    else:
                        one, brow, bbase = oneb, brb_sb, (li - 1) * G
                    for n in range(3):
                        nc.tensor.matmul(
                            g_ps[:, n * H:(n + 1) * H],
                            one[0:1, :],
                            brow[0:1, bbase + n * H:bbase + (n + 1) * H],
                            start=True, stop=False)
                for k in range(4):
                    if li == 0:
                        lhsT = xa[t][:, k * P:(k + 1) * P]
                        wsrc, wbase = w1_sb, k * G
                    else:
                        lhsT = aT[t][:, k * P:(k + 1) * P]
                        wsrc, wbase = wb_sb, ((li - 1) * 4 + k) * G
                    for n in range(3):
                        nc.tensor.matmul(
                            g_ps[:, n * H:(n + 1) * H],
                            lhsT,
                            wsrc[:, wbase + n * H:wbase + (n + 1) * H],
                            start=(k == 0 and not use_opener),
                            stop=(k == 3),
                        )
                return g_ps

            def emit_head(li, t):
                """mm + DVE bias-add (frees PSUM asap) + one tanh."""
                g_ps = emit_mm(li, t, False)
                g_sb = gsbp.tile([P, G], dt.bfloat16, tag="gsb", name="g_sb")
                nc.vector.tensor_add(g_sb[:], g_ps[:],
                                     brep_sb[:, li * G:(li + 1) * G])
                t_all = actp.tile([P, G], dt.bfloat16, tag="tall", name="t_all")
                nc.scalar.activation(t_all[:], g_sb[:], AF.Tanh, scale=0.5)
                return t_all

            def emit_cprime(li, t, t_all):
                """c' = (ti+1)*tg into the pair tile + tanh(c')."""
                hc_t = hcp.tile([P, 2 * H], dt.bfloat16, tag="hc", name="hc_t")
                tp1 = tp1p.tile([P, H], dt.bfloat16, tag="tp1", name="tp1")
                nc.vector.tensor_scalar_add(tp1[:], t_all[:, 0:H], 1.0)
                nc.vector.tensor_mul(hc_t[:, H:2 * H], tp1[:], t_all[:, 1024:G])
                tcl = actp.tile([P, H], dt.bfloat16, tag="tcl", name="tcl")
                nc.scalar.activation(tcl[:], hc_t[:, H:2 * H], AF.Tanh,
                                     scale=0.5)
                return hc_t, tcl

            def emit_tail(li, t, t_all, hc_t, tcl):
                """DVE h' + one paired store per (tile, slot)."""
                tp2 = tp1p.tile([P, H], dt.bfloat16, tag="tp2", name="tp2")
                nc.vector.tensor_scalar_add(tp2[:], t_all[:, H:1024], 1.0)
                nc.vector.tensor_mul(hc_t[:, 0:H], tp2[:], tcl[:])
                if li < 3:
                    hprev[t] = hc_t
                for k, m in enumerate(MT[t]):
                    base = B[t][k]
                    seng = nc.sync if k == 0 else nc.gpsimd
                    seng.dma_start(o_d[li, base:base + m], hc_t[0:m, :])

            def emit_transpose(t):
                """h'(t) -> aT(t) for the next layer (PE transpose, bf16)."""
                h_t = hprev[t]
                pt = pstp.tile([P, H], dt.bfloat16, tag="pst", name="pt")
                for k in range(4):
                    nc.tensor.transpose(pt[:, k * P:(k + 1) * P],
                                        h_t[:, k * P:(k + 1) * P], id_bf[:])
                a_t = aTp.tile([P, H], dt.bfloat16, tag=f"aT{t}", name=f"aT{t}")
                nc.scalar.copy(a_t[:], pt[:])
                aT[t] = a_t
                hprev[t] = None

            # ---- flat software pipeline over u = li*NT + t.  Per-step DVE
            # queue is [bias-add(u) | c'(u-1) | h'(u-2)]: the PSUM-freeing
            # add is never head-blocked behind ops waiting on ACT.
            qa = []                        # (li, t, t_all) awaiting c'
            qb = []                        # (li, t, t_all, hc, tcl) -> h'
            tpend = []                     # tile ids awaiting transpose
            for u in range(4 * NT):
                li, t = u // NT, u % NT
                qa.append((li, t, emit_head(li, t)))
                if len(qa) > 1:
                    al, at, ata = qa.pop(0)
                    hc_t, tcl = emit_cprime(al, at, ata)
                    qb.append((al, at, ata, hc_t, tcl))
                if len(qb) > 1:
                    bl, bt, *rest = qb.pop(0)
                    emit_tail(bl, bt, *rest)
                    if bl < 3:
                        tpend.append(bt)
                if len(tpend) > 1 or (tpend and li == 3):
                    emit_transpose(tpend.pop(0))
            while qa:
                al, at, ata = qa.pop(0)
                hc_t, tcl = emit_cprime(al, at, ata)
                qb.append((al, at, ata, hc_t, tcl))
            while qb:
                bl, bt, *rest = qb.pop(0)
                emit_tail(bl, bt, *rest)
                if bl < 3:
                    tpend.append(bt)
            while tpend:
                emit_transpose(tpend.pop(0))
    nc.compile()
    return nc


# ---------------------------------------------------------------- entry point

def _ensure_axon_hooks():
    try:
        import antenv.axon_hooks  # noqa: F401
        return
    except ImportError:
        pass
    import types
    import contextlib
    import ctypes

    def _build_hook():
        so = "/opt/axon/libaxon_pjrt.so"
        try:
            lib = ctypes.CDLL(so)
        except OSError:
            return None
        if not hasattr(lib, "axon_start_nrt_profile"):
            return None
        lib.axon_start_nrt_profile.argtypes = [
            ctypes.POINTER(ctypes.c_int64), ctypes.c_size_t]
        lib.axon_start_nrt_profile.restype = ctypes.c_int64
        lib.axon_stop_nrt_profile.argtypes = [ctypes.c_char_p]
        lib.axon_stop_nrt_profile.restype = ctypes.c_int64

        @contextlib.contextmanager
        def _hook(output_dir, device_ids):
            import jax
            jax.devices()
            if device_ids:
                ids = (ctypes.c_int64 * len(device_ids))(*device_ids)
                rc = lib.axon_start_nrt_profile(ids, len(device_ids))
            else:
                rc = lib.axon_start_nrt_profile(None, 0)
            if rc != 0:
                raise RuntimeError(f"axon_start_nrt_profile rc={rc}")
            try:
                yield
            finally:
                n = lib.axon_stop_nrt_profile(str(output_dir).encode())
                print(f"ntff profile: {n} file(s) written to {output_dir}",
                      file=sys.stderr)

        return _hook

    box = [None, False]

    def set_axon_ntff_profile_hook(h):
        box[0] = h
        box[1] = True

    def get_axon_ntff_profile_hook():
        if not box[1]:
            box[0] = _build_hook()
            box[1] = True
        return box[0]

    mod = types.ModuleType("antenv.axon_hooks")
    mod.set_axon_ntff_profile_hook = set_axon_ntff_profile_hook
    mod.get_axon_ntff_profile_hook = get_axon_ntff_profile_hook
    import antenv
    sys.modules["antenv.axon_hooks"] = mod
    antenv.axon_hooks = mod


_cache = {}


def kernel(**inputs):
    packed_x = np.asarray(inputs["packed_x"], np.float32)
    bs = np.asarray(inputs["batch_sizes"])

    key = bs.tobytes()
    if key not in _cache:
        plan = _make_plan(bs)
        nc = _build_nc(plan["T_tiles"], plan["MT"], plan["B"],
                       plan["O_alloc"], plan["OBIG"])
        _cache[key] = (plan, nc)
    plan, nc = _cache[key]

    w1, wb, br1, brb, brep = _pack_weights(inputs)
    T_tiles = plan["T_tiles"]

    in_maps = []
    for cc in plan["cores"]:
        x = np.zeros((T_tiles * P, H), np.float32)
        x[:len(cc["src_o"])] = packed_x[cc["src_o"]]
        # [tile, feat-in-chunk(part), kchunk*tok]: 2KB/partition descriptors
        xt = np.ascontiguousarray(
            x.reshape(T_tiles, P, 4, P).transpose(0, 3, 2, 1)
        ).reshape(T_tiles, P, 4 * P)
        in_maps.append({"xt": xt, "w1": w1, "wb": wb,
                        "br1": br1, "brb": brb, "brep": brep})

    from concourse.bass_utils import run_bass_kernel_spmd
    _ensure_axon_hooks()
    res = run_bass_kernel_spmd(nc, in_maps, core_ids=list(range(NCORES)))
    global last_result
    last_result = res

    full = {}
    for jo, nm in enumerate(OUT_NAMES):
        li, is_c = jo // 2, jo % 2
        f = np.zeros((plan["Nout"], H), np.float32)
        for c, cc in enumerate(plan["cores"]):
            slab = np.asarray(res.results[c]["hc"])[li, :, is_c]
            f[cc["glob_rows"]] = slab[cc["slab_rows"]].astype(np.float32) * 0.5
        full[nm] = f

    return (full["h4"], full["h1"], full["c1"], full["h2"], full["c2"],
            full["h3"], full["c3"], full["h4"], full["c4"])


if __name__ == "__main__":
    import reference
    inputs = reference.setup_inputs()
    out = kernel(**{k: np.asarray(v) for k, v in inputs.items()})
    print([o.shape for o in out])
